# revision 17
# baseline (speedup 1.0000x reference)
"""Multi-head linear attention (elu+1 feature map) on 8 TRN2 NeuronCores.

Sharding: core c handles batch b = c//2, sequence half j = c%2 (2048 rows).
Each core computes q/k/v projections + phi + partial kv/z for its rows,
AllReduces kv/z across the (b, j) pair, then computes num/den/ctx and the
output projection for its rows. All matmuls in bf16 (fp32 PSUM accumulate).

Schedule (engine-balanced, collective-overlapped):
  - K1: per pair, project k|v (N=256 chunks), phi via Exp(Scalar) +
    min(GpSimd) + fused merge(Vector); the v PSUM->SBUF move alternates
    Scalar/Vector per group so neither engine gates the ~1.0us group
    cadence.  kv/z accumulate via the [v|ones] N=129 matmul.
  - AllReduce 1 fires after pair 3 (mid-K1), AllReduce 2 at K1 end; both
    carry [128, 4*65] bf16 diagonal kv blocks + z columns.
  - qf runs qc-major after K1; den for pairs 0-3 (rows 0:8 of recip) is
    interleaved per 512-column chunk as soon as the first AllReduce lands;
    den needs only its own pairs thanks to the zero-padded z lhsT.
  - num for pairs 0-3 / first 1024 columns + the first two output-chunk
    partial accumulations (k=0..3) run while AllReduce 2 is in flight;
    den/num for pairs 4-7 follow when it lands, then the O stream.
  - Reciprocal runs directly on the den PSUM with eps folded into the
    activation's immediate bias (den >> eps so placement is irrelevant).
  - recip broadcasts are SBUF->SBUF DMAs on the gpsimd/sync queues.
  - O projection: per-si [128, E] PSUM tiles (bufs=2), evictions on
    Scalar only; y stores on the sync queue.
"""
import numpy as np
import ml_dtypes

B, S, H, Dh = 4, 4096, 16, 64
E = H * Dh
N_CORES = 8
SL = S // 2          # sequence rows per core
NPAIR = H // 2       # head pairs
HALF = NPAIR // 2
EPS = 1e-6

_CACHE = {}


def _build_program():
    import concourse.bacc as bacc
    import concourse.mybir as mybir
    import concourse.tile as tile

    bf16 = mybir.dt.bfloat16
    f32 = mybir.dt.float32
    Act = mybir.ActivationFunctionType
    Alu = mybir.AluOpType

    nc = bacc.Bacc(None, target_bir_lowering=False, num_devices=N_CORES)

    xq = nc.dram_tensor("xqT", [E, SL], bf16, kind="ExternalInput")
    wq_bd = nc.dram_tensor("wq_bd", [NPAIR, 128, 128], bf16, kind="ExternalInput")
    wkv_bd = nc.dram_tensor("wkv_bd", [NPAIR, 128, 256], bf16, kind="ExternalInput")
    wo = nc.dram_tensor("wo", [E, E], bf16, kind="ExternalInput")
    y = nc.dram_tensor("y", [SL, E], f32, kind="ExternalOutput")
    kv_ar = nc.dram_tensor("kv_ar", [128, HALF * 65], bf16)
    kv_ar2 = nc.dram_tensor("kv_ar2", [128, HALF * 65], bf16)

    NCHUNK = SL // 128   # s-chunks per pair (16)
    GC = 4               # chunks per phi/eviction group

    def raw_recip(eng, out_ap, in_ap, bias):
        eng.add_instruction(
            mybir.InstActivation(
                name=nc.get_next_instruction_name(),
                func=Act.Reciprocal,
                ins=[
                    eng.lower_ap(in_ap),
                    mybir.ImmediateValue(dtype=f32, value=bias),
                    mybir.ImmediateValue(dtype=f32, value=1.0),
                    mybir.ImmediateValue(dtype=f32, value=0.0),
                ],
                outs=[eng.lower_ap(out_ap)],
            )
        )

    with tile.TileContext(nc) as tc:
        with (
            tc.tile_pool(name="persist", bufs=1) as persist,
            tc.tile_pool(name="xp", bufs=1) as xp,
            tc.tile_pool(name="kvsb", bufs=2) as kvsb,
            tc.tile_pool(name="tmp", bufs=3) as tmp,
            tc.tile_pool(name="rbcp", bufs=6) as rbcp,
            tc.tile_pool(name="outp", bufs=2) as outp,
            tc.tile_pool(name="dram", bufs=1, space="DRAM") as dram,
        ):
            # ---- weights / constants ----
            # load order: pair-0 weights, pair-0 x (split), then the rest,
            # so the first projection matmul can start ~1MB into the stream.
            wkv_sb = persist.tile([128, NPAIR, 256], bf16)
            wkv_re = wkv_bd.rearrange("p k m -> k p m")
            nc.sync.dma_start(out=wkv_sb[:, 0, :], in_=wkv_re[:, 0, :])
            xTs = []
            for p in range(NPAIR):
                xT = xp.tile([128, SL], bf16, tag=f"xT{p}")
                if p == 0:
                    nc.sync.dma_start(out=xT[:, 0:1024], in_=xq[0:128, 0:1024])
                    nc.sync.dma_start(out=xT[:, 1024:SL], in_=xq[0:128, 1024:SL])
                    nc.sync.dma_start(
                        out=wkv_sb[:, 1:NPAIR, :], in_=wkv_re[:, 1:NPAIR, :]
                    )
                else:
                    nc.sync.dma_start(out=xT[:], in_=xq[p * 128:(p + 1) * 128, :])
                xTs.append(xT)
            wq_sb = persist.tile([128, NPAIR, 128], bf16)
            nc.gpsimd.dma_start(out=wq_sb[:], in_=wq_bd.rearrange("p k m -> k p m"))
            wo_sb = persist.tile([128, NPAIR, E], bf16)
            nc.gpsimd.dma_start(
                out=wo_sb[:], in_=wo.rearrange("(k p) n -> p k n", p=128)
            )
            qfT = persist.tile([128, NPAIR, SL], bf16)
            ctxT = persist.tile([128, NPAIR, SL], bf16)
            recip_a = persist.tile([8, SL], bf16)
            recip_b = persist.tile([8, SL], bf16)
            recip_f32 = persist.tile([8, SL], f32)
            recip_dram = dram.tile([16, SL], bf16)

            # kv/z staging buffers (diag blocks per pair; zeros elsewhere)
            kvbd_a = persist.tile([128, HALF, 128], bf16)
            nc.vector.memset(kvbd_a[:], 0.0)
            kvbd_b = persist.tile([128, HALF, 128], bf16)
            nc.vector.memset(kvbd_b[:], 0.0)
            zbd_a = persist.tile([128, HALF, 8], bf16)
            nc.vector.memset(zbd_a[:], 0.0)
            zbd_b = persist.tile([128, HALF, 8], bf16)
            nc.vector.memset(zbd_b[:], 0.0)

            HKV = HALF * 65
            kv_in = dram.tile([128, HKV], bf16)
            kv_in2 = dram.tile([128, HKV], bf16)
            groups = [[0, 1], [2, 3], [4, 5], [6, 7]]

            def stage_half(ar, kvbd, zbd):
                # diag kv blocks: one DMA per head row-half
                nc.gpsimd.dma_start(
                    out=kvbd[0:64, :, 0:64],
                    in_=ar.rearrange("q (p c) -> q p c", c=65)[0:64, :, 0:64],
                )
                nc.sync.dma_start(
                    out=kvbd[64:128, :, 64:128],
                    in_=ar.rearrange("q (p c) -> q p c", c=65)[64:128, :, 0:64],
                )
                for p in range(HALF):
                    nc.gpsimd.dma_start(
                        out=zbd[0:64, p, 2 * p:2 * p + 1],
                        in_=ar[0:64, p * 65 + 64:p * 65 + 65],
                    )
                    nc.sync.dma_start(
                        out=zbd[64:128, p, 2 * p + 1:2 * p + 2],
                        in_=ar[64:128, p * 65 + 64:p * 65 + 65],
                    )

            # ---- phase K1: kf/v (s-major) + kv/z for every pair ----
            with (
                tc.tile_pool(name="ps_kvp", bufs=3, space="PSUM") as ps_kvp,
                tc.tile_pool(name="ps_kv", bufs=2, space="PSUM") as ps_kv,
            ):
                for p in range(NPAIR):
                    xT = xTs[p]
                    kf = kvsb.tile([128, NCHUNK, 128], bf16, tag="kf")
                    vz = kvsb.tile([128, NCHUNK, 129], bf16, tag="vz")
                    nc.vector.memset(vz[:, :, 128:129], 1.0)
                    kvzacc = ps_kv.tile([128, 129], f32, tag="kvz")

                    def proj_group(g):
                        kvps = ps_kvp.tile([128, GC, 256], f32, tag="kvps")
                        for c in range(GC):
                            i = g * GC + c
                            nc.tensor.matmul(
                                kvps[:, c, :],
                                lhsT=xT[:, i * 128:(i + 1) * 128],
                                rhs=wkv_sb[:, p, :],
                                start=True, stop=True,
                            )
                        return kvps

                    def phi_group(g, kvps):
                        cs = slice(g * GC, (g + 1) * GC)
                        E1 = tmp.tile([128, GC, 128], bf16, tag="E1")
                        nc.scalar.activation(E1[:], kvps[:, :, 0:128], Act.Exp)
                        Em = tmp.tile([128, GC, 128], bf16, tag="R1")
                        nc.vector.tensor_scalar_min(Em[:], E1[:], 1.0)
                        nc.vector.scalar_tensor_tensor(
                            kf[:, cs, :], kvps[:, :, 0:128], 1.0, Em[:],
                            Alu.add, Alu.max,
                        )
                        nc.scalar.copy(vz[:, cs, 0:128], kvps[:, :, 128:256])

                    def acc_group(g):
                        for c in range(GC):
                            i = g * GC + c
                            nc.tensor.matmul(
                                kvzacc[:],
                                lhsT=kf[:, i, :], rhs=vz[:, i, :],
                                start=(i == 0), stop=(i == NCHUNK - 1),
                            )

                    # software pipeline: P0 P1 A0 P2 A1 P3 A2 A3
                    ps = [proj_group(0)]
                    phi_group(0, ps[0])
                    ps.append(proj_group(1))
                    phi_group(1, ps[1])
                    acc_group(0)
                    ps.append(proj_group(2))
                    phi_group(2, ps[2])
                    acc_group(1)
                    ps.append(proj_group(3))
                    phi_group(3, ps[3])
                    acc_group(2)
                    acc_group(3)

                    kvst = outp.tile([128, 65], bf16, tag="kvst")
                    nc.vector.tensor_copy(kvst[0:64, 0:64], kvzacc[0:64, 0:64])
                    nc.vector.tensor_copy(
                        kvst[64:128, 0:64], kvzacc[64:128, 64:128]
                    )
                    nc.vector.tensor_copy(kvst[:, 64:65], kvzacc[:, 128:129])
                    tgt = kv_in if p < HALF else kv_in2
                    nc.sync.dma_start(
                        out=tgt[:, (p % HALF) * 65:(p % HALF + 1) * 65],
                        in_=kvst[:],
                    )
                    if p == HALF - 1:
                        # first-half AllReduce fires mid-K1; latency hides
                        # under the second half of K1.
                        nc.gpsimd.collective_compute(
                            "AllReduce", Alu.add, replica_groups=groups,
                            ins=[kv_in[:]], outs=[kv_ar[:]],
                        )
                        stage_half(kv_ar, kvbd_a, zbd_a)

            # ---- AllReduce for pairs 4-7; staging rides idle queues ----
            nc.gpsimd.collective_compute(
                "AllReduce", Alu.add, replica_groups=groups,
                ins=[kv_in2[:]], outs=[kv_ar2[:]],
            )
            stage_half(kv_ar2, kvbd_b, zbd_b)

            def den_chunk(denps, zbd, plist, q5):
                qs = slice(q5 * 512, (q5 + 1) * 512)
                for i, p in enumerate(plist):
                    nc.tensor.matmul(
                        denps[:], lhsT=zbd[:, i, :], rhs=qfT[:, p, qs],
                        start=(i == 0), stop=(i == len(plist) - 1),
                    )

            def emit_num(ps_num, p, blk, kvbd):
                # 512-wide block blk (0..3) of pair p: rbc broadcast + matmul
                # + fused divide-by-den eviction into ctxT.
                hs = slice(blk * 512, (blk + 1) * 512)
                rbc = rbcp.tile([128, 512], bf16, tag="rbc")
                nc.gpsimd.dma_start(
                    out=rbc[0:64, :],
                    in_=recip_dram[2 * p:2 * p + 1, hs].to_broadcast([64, 512]),
                )
                nc.sync.dma_start(
                    out=rbc[64:128, :],
                    in_=recip_dram[2 * p + 1:2 * p + 2, hs].to_broadcast([64, 512]),
                )
                nps = ps_num.tile([128, 512], f32, tag="nps")
                nc.tensor.matmul(
                    nps[:], lhsT=kvbd[:, p % HALF, :], rhs=qfT[:, p, hs],
                    start=True, stop=True,
                )
                nc.vector.tensor_tensor(ctxT[:, p, hs], nps[:], rbc[:], Alu.mult)

            def o_mms(ops, si, k, start, stop):
                ss = slice(si * 128, (si + 1) * 128)
                nc.tensor.matmul(
                    ops[:, 0:512], lhsT=ctxT[:, k, ss], rhs=wo_sb[:, k, 0:512],
                    start=start, stop=stop,
                )
                nc.tensor.matmul(
                    ops[:, 512:E], lhsT=ctxT[:, k, ss], rhs=wo_sb[:, k, 512:E],
                    start=start, stop=stop,
                )

            def o_evict(ops, si):
                ss = slice(si * 128, (si + 1) * 128)
                ysb = outp.tile([128, E], f32, tag="ysb")
                nc.scalar.copy(ysb[:, 0:512], ops[:, 0:512])
                nc.scalar.copy(ysb[:, 512:E], ops[:, 512:E])
                nc.sync.dma_start(out=y[ss, 0:512], in_=ysb[:, 0:512])
                nc.sync.dma_start(out=y[ss, 512:E], in_=ysb[:, 512:E])

            with (
                tc.tile_pool(name="ps_den", bufs=2, space="PSUM") as ps_den,
                tc.tile_pool(name="ps_num", bufs=2, space="PSUM") as ps_num,
            ):
                # ---- qf (qc-major) with den-A interleaved per q5 ----
                with tc.tile_pool(name="ps_q", bufs=2, space="PSUM") as ps_q:
                    tc.tile_set_cur_wait(0.30)
                    for qc in range(2):
                        qs = slice(qc * 1024, (qc + 1) * 1024)
                        for p in range(NPAIR):
                            xT = xTs[p]
                            qps = ps_q.tile([128, 1024], f32, tag="qps")
                            nc.tensor.matmul(
                                qps[:, 0:512], lhsT=wq_sb[:, p, :],
                                rhs=xT[:, qc * 1024:qc * 1024 + 512],
                                start=True, stop=True,
                            )
                            nc.tensor.matmul(
                                qps[:, 512:1024], lhsT=wq_sb[:, p, :],
                                rhs=xT[:, qc * 1024 + 512:(qc + 1) * 1024],
                                start=True, stop=True,
                            )
                            qE = tmp.tile([128, 1024], bf16, tag="E1")
                            nc.scalar.activation(qE[:], qps[:], Act.Exp)
                            qM = tmp.tile([128, 1024], bf16, tag="R1")
                            if qc == 1:
                                # relu scheme: Scalar does the second PSUM
                                # read; Vector's merge runs at bf16 2x rate.
                                nc.scalar.activation(qM[:], qps[:], Act.Relu)
                                nc.vector.scalar_tensor_tensor(
                                    qfT[:, p, qs], qE[:], 1.0, qM[:],
                                    Alu.min, Alu.add,
                                )
                            else:
                                nc.vector.tensor_scalar_min(qM[:], qE[:], 1.0)
                                nc.vector.scalar_tensor_tensor(
                                    qfT[:, p, qs], qps[:], 1.0, qM[:],
                                    Alu.add, Alu.max,
                                )
                        # den-A for this qc's two q5 chunks (pairs 0-3 only;
                        # their den rows are final thanks to zero-padded z).
                        tc.tile_set_cur_wait(0.305 + qc * 0.005)
                        for q5 in (2 * qc, 2 * qc + 1):
                            qs = slice(q5 * 512, (q5 + 1) * 512)
                            denps = ps_den.tile([8, 512], f32, tag="den")
                            den_chunk(denps, zbd_a, list(range(HALF)), q5)
                            nc.vector.reciprocal_approx_fast(
                                out=recip_f32[:, qs], in_=denps[:]
                            )
                            nc.vector.tensor_copy(
                                recip_a[:, qs], recip_f32[:, qs]
                            )
                            nc.gpsimd.dma_start(
                                out=recip_dram[0:8, qs], in_=recip_a[:, qs]
                            )
                    # ---- num-A for qc0 blocks; AllReduce 2 still in flight --
                    tc.tile_set_cur_wait(0.315)
                    for p in range(HALF):
                        for blk in range(2):
                            emit_num(ps_num, p, blk, kvbd_a)

                # ---- O stream; den/num B when AllReduce 2 lands ----
                with tc.tile_pool(name="ps_o", bufs=2, space="PSUM") as ps_o:
                    tc.tile_set_cur_wait(0.32)
                    o_tiles = {}
                    for si in (0, 1):
                        ops = ps_o.tile([128, E], f32, tag="ops")
                        o_tiles[si] = ops
                        for k in range(HALF):
                            o_mms(ops, si, k, start=(k == 0), stop=False)

                    tc.tile_set_cur_wait(0.33)
                    for q5 in range(4):
                        qs = slice(q5 * 512, (q5 + 1) * 512)
                        denps = ps_den.tile([8, 512], f32, tag="den")
                        den_chunk(
                            denps, zbd_b, list(range(HALF, NPAIR)), q5
                        )
                        nc.vector.reciprocal_approx_fast(
                            out=recip_f32[:, qs], in_=denps[:]
                        )
                        nc.vector.tensor_copy(
                            recip_b[:, qs], recip_f32[:, qs]
                        )
                        nc.gpsimd.dma_start(
                            out=recip_dram[8:16, qs], in_=recip_b[:, qs]
                        )
                    # num-B qc0, then num-A qc1, then num-B qc1
                    tc.tile_set_cur_wait(0.335)
                    for p in range(HALF, NPAIR):
                        for blk in range(2):
                            emit_num(ps_num, p, blk, kvbd_b)
                    for p in range(HALF):
                        for blk in range(2, 4):
                            emit_num(ps_num, p, blk, kvbd_a)
                    for p in range(HALF, NPAIR):
                        for blk in range(2, 4):
                            emit_num(ps_num, p, blk, kvbd_b)

                    tc.tile_set_cur_wait(0.34)
                    for si in (0, 1):
                        ops = o_tiles[si]
                        for k in range(HALF, NPAIR):
                            o_mms(ops, si, k, start=False, stop=(k == NPAIR - 1))
                        o_evict(ops, si)
                    for si in range(2, NCHUNK):
                        tc.tile_set_cur_wait(0.34 + si * 0.001)
                        ops = ps_o.tile([128, E], f32, tag="ops")
                        for k in range(NPAIR):
                            o_mms(ops, si, k, start=(k == 0), stop=(k == NPAIR - 1))
                        o_evict(ops, si)

    nc.compile()
    return nc


def _get_program():
    if "nc" not in _CACHE:
        _CACHE["nc"] = _build_program()
    return _CACHE["nc"]


def _host_prep(query, Wq, Wk, Wv, Wo):
    bf16 = ml_dtypes.bfloat16
    q_bf = np.ascontiguousarray(query.astype(bf16))
    wq_bd = np.zeros((NPAIR, 128, 128), dtype=bf16)
    wkv_bd = np.zeros((NPAIR, 128, 256), dtype=bf16)
    for p in range(NPAIR):
        wq_bd[p, 0:64, 0:64] = Wq[2 * p]
        wq_bd[p, 64:128, 64:128] = Wq[2 * p + 1]
        wkv_bd[p, 0:64, 0:64] = Wk[2 * p]
        wkv_bd[p, 64:128, 64:128] = Wk[2 * p + 1]
        wkv_bd[p, 0:64, 128:192] = Wv[2 * p]
        wkv_bd[p, 64:128, 192:256] = Wv[2 * p + 1]
    wo_bf = np.ascontiguousarray(Wo.astype(bf16))
    in_maps = []
    for c in range(N_CORES):
        b, j = divmod(c, 2)
        in_maps.append({
            "xqT": np.ascontiguousarray(q_bf[b, j * SL:(j + 1) * SL, :].T),
            "wq_bd": wq_bd,
            "wkv_bd": wkv_bd,
            "wo": wo_bf,
        })
    return in_maps


def kernel(query, Wq, Wk, Wv, Wo):
    from concourse.bass_utils import run_bass_kernel_spmd

    nc = _get_program()
    in_maps = _host_prep(query, Wq, Wk, Wv, Wo)
    res = run_bass_kernel_spmd(nc, in_maps, list(range(N_CORES)))
    out = np.empty((B, S, E), dtype=np.float32)
    for c in range(N_CORES):
        b, j = divmod(c, 2)
        out[b, j * SL:(j + 1) * SL, :] = res.results[c]["y"]
    return out
